# revision 17
# baseline (speedup 1.0000x reference)
"""Bass/Trainium2 kernel for the CIFlow loss function (v3).

Contract: kernel(**inputs) takes the FULL unsharded inputs (as produced by
setup_inputs()) and returns the full scalar output, distributing work over
8 NeuronCores internally via run_bass_kernel_spmd.

Device (per core, data-parallel over 32 graphs / 16384 nodes), fp8(e4m3)
inputs with f32 PSUM accumulation:
  - one-hot cluster masks expanded on device from the sampled assignment
    column (DVE is_equal against k), so only 1 index column is streamed.
  - feature family: per-graph segment matmuls with the WIDE operand
    stationary: psA[75, g, 10] += [H | S^2*64 | rownorm2/16]^T @ onehot,
    one PSUM accumulation group per graph (4 chunks of 128 rows).
  - prototype family: psB[65, 10] += [E | 1]^T @ (Q*8) over all chunks.
  - qmax: DVE free-axis max-reduce over the Q columns.
Host: PRNG-exact cluster sampling (jax categorical, key 42), per-segment
counts (bincount, as in the v1 baseline), sparse edge term, and the tiny
scalar reductions combining the device outputs.
"""

import numpy as np

B, M, K, D, C = 256, 512, 10, 64, 2
N = 131072
NNZ = 2097152
LAMBDA_2, LAMBDA_CON, LAMBDA_FEA, LAMBDA_PROTO = 0.1, 1.0, 1.0, 0.1

NC = 8
N_SH = N // NC          # 16384 rows per core
G_SH = B // NC          # 32 graphs per core
CHUNKS = N_SH // 128    # 128 chunks of 128 rows
NH = 2                  # mov halves
HCH = CHUNKS // NH      # 64 chunks per half

# sta DMA pieces (in chunks): 32+32+32+24+8 — a small final piece keeps the
# post-stream dependency tail short.
STA_PIECES = [32, 32, 32, 24, 8]

SCALE_S2 = 64.0         # S^2 scaled into fp8 range
SCALE_RN = 1.0 / 16.0   # rownorm2 scaled down
SCALE_Q = 8.0           # Q scaled up

OUT_MODE = "plain"         # "kv" (SWDGE writeback) or "plain" (DMACopy)

STA_W = 140             # [H(64) | S2(10) | rn2(1) | E(64) | 1(1)]
MOV_W = 11              # [assign(1) | Q(10)]

_CACHE = {}


def _build_program():
    import concourse.bass as bass
    import concourse.bacc as bacc
    import concourse.tile as tile
    from concourse import mybir

    f32 = mybir.dt.float32
    f8 = mybir.dt.float8e4
    nc = bacc.Bacc("TRN2", target_bir_lowering=False, debug=False, num_devices=NC)

    sta_d = [nc.dram_tensor(f"sta{p}_in", [128, w, STA_W], f8,
                            kind="ExternalInput").ap()
             for p, w in enumerate(STA_PIECES)]
    mov_d = [nc.dram_tensor(f"mov{h}_in", [128, MOV_W, HCH], f8,
                            kind="ExternalInput").ap() for h in range(NH)]

    # single packed output, shipped via a prepared kv_writeback (SWDGE) so
    # the post-compute chain is just trigger+transfer+sem: cols 0:320 feat
    # ([75, 32, 10]), 320:330 qmax, 330:340 proto ([65, 10]), rest pad
    if OUT_MODE == "kv":
        out_d = nc.dram_tensor("out", [1, 128, 1, 512], f32, kind="ExternalOutput").ap()
    else:
        out_d = nc.dram_tensor("out", [128, 512], f32, kind="ExternalOutput").ap()

    PS = bass.MemorySpace.PSUM

    # chunk -> sta piece mapping
    piece_of = []
    for p, w in enumerate(STA_PIECES):
        piece_of += [(p, i) for i in range(w)]

    with tile.TileContext(nc) as tc:
        with (
            tc.tile_pool(name="big", bufs=1) as big,
            tc.tile_pool(name="work", bufs=1) as work,
            tc.tile_pool(name="ps", bufs=1, space=PS) as ps,
        ):
            sta_sb = [big.tile([128, w, STA_W], f8, tag=f"sta{p}", name=f"sta_sb{p}")
                      for p, w in enumerate(STA_PIECES)]
            mov_sb = [big.tile([128, MOV_W, HCH], f8, tag=f"mov{h}", name=f"mov_sb{h}")
                      for h in range(NH)]
            nc.sync.dma_start(sta_sb[0][:], sta_d[0][:])
            nc.scalar.dma_start(mov_sb[0][:], mov_d[0][:])
            nc.scalar.dma_start(mov_sb[1][:], mov_d[1][:])
            nc.sync.dma_start(sta_sb[1][:], sta_d[1][:])
            nc.sync.dma_start(sta_sb[2][:], sta_d[2][:])
            nc.sync.dma_start(sta_sb[3][:], sta_d[3][:])
            nc.sync.dma_start(sta_sb[4][:], sta_d[4][:])

            stage = work.tile([128, 1, 1, 512], f32, tag="stage")
            nc.gpsimd.memset(stage[:], 0.0)
            if OUT_MODE == "kv":
                kvidx = work.tile([128, 1], mybir.dt.int32, tag="kvidx")
                nc.gpsimd.memset(kvidx[:], 0)
                kv_sem = nc.alloc_semaphore("kv_dma")
                nc.gpsimd.kv_writeback(out_d[:], stage[:], kvidx[:],
                                       prepare_only=True, sem=kv_sem)

            # on-device one-hot expansion: oh[k, ch] = (assign == k)
            oh_sb = [work.tile([128, 10, HCH], f8, tag=f"oh{h}", name=f"oh_sb{h}")
                     for h in range(NH)]
            for h in range(NH):
                for k in range(10):
                    nc.vector.tensor_scalar(
                        oh_sb[h][:, k, :], mov_sb[h][:, 0, :], float(k), None,
                        op0=mybir.AluOpType.is_equal)

            # one PSUM bank per 8-graph quarter so the early drain copies
            # never share a bank with in-flight matmul writes
            psA = [ps.tile([75, 8, 10], f32, tag=f"psA{q}", name=f"psA{q}")
                   for q in range(4)]
            psB = ps.tile([65, 10], f32, tag="psB")

            for c in range(CHUNKS):
                p, cq = piece_of[c]
                h, ch = divmod(c, HCH)
                g = c // 4
                nc.tensor.matmul(psA[g // 8][:, g % 8, :],
                                 sta_sb[p][:, cq, 0:75],
                                 oh_sb[h][:, :, ch],
                                 start=(c % 4 == 0), stop=(c % 4 == 3))
                nc.tensor.matmul(psB[:],
                                 sta_sb[p][:, cq, 75:140],
                                 mov_sb[h][:, 1:11, ch],
                                 start=(c == 0), stop=(c == CHUNKS - 1))
                # drain finished quarters out of PSUM early (ACT engine), so
                # only the last small copy sits on the tail
                if c % 32 == 31:
                    q = c // 32
                    nc.scalar.copy(stage[0:75, 0, 0, q * 80:(q + 1) * 80], psA[q][:])

            # qmax over all chunks: free-axis max-reduce per mov half, then
            # combine the halves straight into the staging tile.
            rmax = [work.tile([128, 10, 1], f32, tag=f"rmax{h}", name=f"rmax{h}")
                    for h in range(NH)]
            for h in range(NH):
                nc.vector.tensor_reduce(rmax[h][:], mov_sb[h][:, 1:11, :],
                                        axis=mybir.AxisListType.X,
                                        op=mybir.AluOpType.max)
            nc.vector.tensor_tensor(stage[:, 0, 0, 320:330], rmax[0][:, :, 0],
                                    rmax[1][:, :, 0], op=mybir.AluOpType.max)
            nc.vector.tensor_copy(stage[0:65, 0, 0, 330:340], psB[:])

            if OUT_MODE == "kv":
                nc.gpsimd.trigger_dma(count=None)
            else:
                nc.sync.dma_start(out_d[:], stage[:, 0, 0, :])

    # Tile's epilogue gates on its DMASW-lane semaphore for the kv prep, but
    # the DMA-completion sem baked into the descriptor is the one passed via
    # sem=. Retarget the prep's completion update to the DMASW sem so the
    # increment lands where the epilogue (and data consumers) wait.
    fn = nc.m.functions[0]
    dmasw = None
    kv_inst = None
    if OUT_MODE != "kv":
        nc.compile()
        return nc
    for b in fn.blocks:
        for inst in b.instructions:
            if type(inst).__name__ == "InstKVWritebackAnt":
                kv_inst = inst
            si = inst.sync_info
            if si is not None:
                for w in (si.on_wait or []):
                    if w.ant_name and w.ant_name.startswith("DMASW") \
                            and w.wait_value == 16:
                        dmasw = (w.id, w.ant_name)
    assert kv_inst is not None and dmasw is not None, (kv_inst, dmasw)
    u0 = kv_inst.sync_info.on_update[0]
    assert u0.ant_name == "kv_dma", u0
    u0.id, u0.ant_name = dmasw

    nc.compile()
    return nc


def _get_program():
    if "nc" not in _CACHE:
        _CACHE["nc"] = _build_program()
    return _CACHE["nc"]


def _host_assign(S):
    """Reproduce jax.random.categorical(key(42), log(S+1e-30)) exactly."""
    import jax
    import jax.numpy as jnp
    cpu = jax.devices("cpu")[0]
    with jax.default_device(cpu):
        a = jax.random.categorical(
            jax.random.key(42), jnp.log(jnp.asarray(S) + 1e-30), axis=-1)
        return np.asarray(a).astype(np.int32)


def _log_softmax(x):
    m = x.max(axis=-1, keepdims=True)
    e = x - m
    return e - np.log(np.exp(e).sum(axis=-1, keepdims=True))


def kernel(Q, E, ind_positive_sample, S, H, L_rows, L_cols, L_vals, batch,
           pred1, pred2, labels):
    import ml_dtypes
    f8 = ml_dtypes.float8_e4m3

    Q = np.asarray(Q, dtype=np.float32)
    E = np.asarray(E, dtype=np.float32)
    S = np.asarray(S, dtype=np.float32)
    H = np.asarray(H, dtype=np.float32)
    L_rows = np.asarray(L_rows)
    L_cols = np.asarray(L_cols)
    L_vals = np.asarray(L_vals, dtype=np.float32)
    pred1 = np.asarray(pred1, dtype=np.float32)
    pred2 = np.asarray(pred2, dtype=np.float32)
    labels = np.asarray(labels).astype(np.int64)

    # host index preprocessing
    assign = _host_assign(S)                        # [N] int32
    Qf = Q.reshape(N, K)
    Ef = E.reshape(N, D)
    rownorm2 = np.einsum('nd,nd->n', H, H)          # [N] f32

    # fp8 staging: sta = [H | S2*64 | rn2/16 | E | 1], mov = [assign | Q*8]
    sta8 = np.empty((N, STA_W), dtype=f8)
    sta8[:, 0:64] = H.astype(f8)
    sta8[:, 64:74] = (S * S * SCALE_S2).astype(f8)
    sta8[:, 74] = (rownorm2 * SCALE_RN).astype(f8)
    sta8[:, 75:139] = Ef.astype(f8)
    sta8[:, 139] = f8(1.0)

    mov8 = np.empty((N, MOV_W), dtype=f8)
    mov8[:, 0] = assign.astype(f8)
    mov8[:, 1:11] = (Qf * SCALE_Q).astype(f8)

    in_maps = []
    for cid in range(NC):
        r0 = cid * N_SH
        m = {}
        c0 = 0
        for p, w in enumerate(STA_PIECES):
            blk = sta8[r0 + c0 * 128: r0 + (c0 + w) * 128]
            m[f"sta{p}_in"] = np.ascontiguousarray(
                blk.reshape(w, 128, STA_W).transpose(1, 0, 2))
            c0 += w
        for h in range(NH):
            blk = mov8[r0 + h * HCH * 128: r0 + (h + 1) * HCH * 128]
            m[f"mov{h}_in"] = np.ascontiguousarray(
                blk.reshape(HCH, 128, MOV_W).transpose(1, 2, 0))
        in_maps.append(m)

    nc = _get_program()
    from concourse.bass_utils import run_bass_kernel_spmd
    res = run_bass_kernel_spmd(nc, in_maps, core_ids=list(range(NC)))
    outs = res.results
    _CACHE["last_exec_time_ns"] = res.exec_time_ns

    # ---- reassemble device outputs ----
    bvec = np.asarray(batch).astype(np.int64)
    counts = np.bincount(bvec * K + assign, minlength=B * K).reshape(B, K)
    counts = counts.astype(np.float32)
    sums = np.zeros((B, K, D), dtype=np.float32)
    colnorm2 = np.zeros((B, K), dtype=np.float32)
    rn2s = np.zeros((B, K), dtype=np.float32)
    proto_sum = np.zeros((K, D), dtype=np.float32)
    q_count = np.zeros((K,), dtype=np.float32)
    qmax = np.full((K,), -np.inf, dtype=np.float32)
    for cid in range(NC):
        out = np.asarray(outs[cid]["out"], dtype=np.float32).reshape(128, 512)
        ft = out[0:75, 0:320].reshape(75, G_SH, 10)
        g0 = cid * G_SH
        sums[g0:g0 + G_SH] = ft[0:64].transpose(1, 2, 0)
        colnorm2[g0:g0 + G_SH] = ft[64:74].sum(axis=2).T / SCALE_S2
        rn2s[g0:g0 + G_SH] = ft[74] / SCALE_RN
        proto_sum += out[0:64, 330:340].T / SCALE_Q
        q_count += out[64, 330:340] / SCALE_Q
        qmax = np.maximum(qmax, out[:, 320:330].max(axis=0) / SCALE_Q)

    # ---- loss_1 / loss_2 ----
    ls1 = _log_softmax(pred1)
    loss_1 = -np.mean(ls1[np.arange(B), labels])
    ls2 = _log_softmax(pred2)
    ce2 = -ls2[np.arange(B), labels]
    mask = np.asarray(ind_positive_sample).astype(np.float32)
    npos = mask.sum()
    loss_2 = LAMBDA_2 * (float((mask * ce2).sum()) / max(npos, 1.0) if npos > 0 else 0.0)

    # ---- connectivity ----
    colnorm = np.sqrt(colnorm2)
    S_n = S / (colnorm[bvec] + 1e-5)
    loss_sp = 0.0
    CH = 1 << 19
    for i in range(0, NNZ, CH):
        r = L_rows[i:i + CH].astype(np.int64)
        c = L_cols[i:i + CH].astype(np.int64)
        v = L_vals[i:i + CH]
        loss_sp += float((v * np.einsum('ek,ek->e', S_n[r], S_n[c])).sum())
    ss = S_n.T @ S_n
    i_s = np.eye(K, dtype=np.float32) * B
    loss_ortho = float(np.sqrt(((ss - i_s) ** 2).sum()))
    con = LAMBDA_CON * (loss_sp + loss_ortho) / B

    # ---- feature loss ----
    cmax = np.maximum(counts, 1.0)
    means = sums / cmax[..., None]
    sq_tot = rn2s - (2.0 * means * sums).sum(-1) + counts * (means * means).sum(-1)
    fd = sq_tot / D
    feature_loss = float(np.where(counts > 0, fd / cmax, 0.0).sum())
    pd = ((means[:, :, None, :] - means[:, None, :, :]) ** 2).mean(axis=-1)
    c_g = 0.5 * pd.sum(axis=(1, 2))
    center = 0.0
    for i in range(B):
        center = (center - float(c_g[i])) / (K - 1)
    fea = LAMBDA_FEA * (feature_loss + center) / B

    # ---- prototype loss ----
    loss1 = float(np.mean(1.0 - qmax))
    proto = proto_sum / (q_count + 0.1)[:, None]
    proto = proto / (np.linalg.norm(proto, axis=1) + 1e-15)[:, None]
    pdist = ((proto[:, None, :] - proto[None, :, :]) ** 2).mean(axis=-1)
    center_loss = -0.5 * float(pdist.sum()) / (K * (K - 1) / 2)
    proto_l = LAMBDA_PROTO * (loss1 + center_loss)

    total = loss_1 + loss_2 + con + fea + proto_l
    return np.float32(total)


# revision 26
# speedup vs baseline: 1.1459x; 1.1459x over previous
"""Bass/Trainium2 kernel for the CIFlow loss function (v3).

Contract: kernel(**inputs) takes the FULL unsharded inputs (as produced by
setup_inputs()) and returns the full scalar output, distributing work over
8 NeuronCores internally via run_bass_kernel_spmd.

Device (per core, data-parallel over 32 graphs / 16384 nodes), fp8(e4m3)
inputs with f32 PSUM accumulation:
  - one-hot cluster masks expanded on device from the sampled assignment
    column (DVE is_equal against k), so only 1 index column is streamed.
  - feature family: per-graph segment matmuls with the WIDE operand
    stationary: psA[75, g, 10] += [H | S^2*64 | rownorm2/16]^T @ onehot,
    one PSUM accumulation group per graph (4 chunks of 128 rows).
  - prototype family: psB[65, 10] += [E | 1]^T @ (Q*8) over all chunks.
  - qmax: DVE free-axis max-reduce over the Q columns.
Host: PRNG-exact cluster sampling (jax categorical, key 42), per-segment
counts (bincount, as in the v1 baseline), sparse edge term, and the tiny
scalar reductions combining the device outputs.
"""

import numpy as np

B, M, K, D, C = 256, 512, 10, 64, 2
N = 131072
NNZ = 2097152
LAMBDA_2, LAMBDA_CON, LAMBDA_FEA, LAMBDA_PROTO = 0.1, 1.0, 1.0, 0.1

NC = 8
N_SH = N // NC          # 16384 rows per core
G_SH = B // NC          # 32 graphs per core
CHUNKS = N_SH // 128    # 128 chunks of 128 rows
NH = 2                  # mov halves
HCH = CHUNKS // NH      # 64 chunks per half

# sta DMA pieces (in chunks): 32+32+32+24+8 — a small final piece keeps the
# post-stream dependency tail short.
STA_PIECES = [32, 32, 32, 24, 8]

SCALE_S2 = 64.0         # S^2 scaled into fp8 range
SCALE_RN = 1.0 / 16.0   # rownorm2 scaled down
SCALE_Q = 8.0           # Q scaled up

OUT_MODE = "kv"         # "kv" (SWDGE writeback) or "plain" (DMACopy)

STA_W = 140             # [H(64) | S2(10) | rn2(1) | E(64) | 1(1)]
MOV_W = 11              # [assign(1) | Q(10)]

_CACHE = {}


def _build_program():
    import concourse.bass as bass
    import concourse.bacc as bacc
    import concourse.tile as tile
    from concourse import mybir

    f32 = mybir.dt.float32
    f8 = mybir.dt.float8e4
    nc = bacc.Bacc("TRN2", target_bir_lowering=False, debug=False, num_devices=NC)

    sta_d = [nc.dram_tensor(f"sta{p}_in", [128, w, STA_W], f8,
                            kind="ExternalInput").ap()
             for p, w in enumerate(STA_PIECES)]
    mov_d = [nc.dram_tensor(f"mov{h}_in", [128, MOV_W, HCH], f8,
                            kind="ExternalInput").ap() for h in range(NH)]

    # single packed output, shipped via a prepared kv_writeback (SWDGE) so
    # the post-compute chain is just trigger+transfer+sem: cols 0:320 feat
    # ([75, 32, 10]), 320:330 qmax, 330:340 proto ([65, 10]), rest pad
    if OUT_MODE == "kv":
        # bf16-typed bitcast of the f32 payload: the kv ucode moves bytes
        out_d = nc.dram_tensor("out", [1, 128, 1, 1024], mybir.dt.bfloat16,
                               kind="ExternalOutput").ap()
    else:
        out_d = nc.dram_tensor("out", [128, 512], f32, kind="ExternalOutput").ap()

    PS = bass.MemorySpace.PSUM

    # chunk -> sta piece mapping
    piece_of = []
    for p, w in enumerate(STA_PIECES):
        piece_of += [(p, i) for i in range(w)]

    with tile.TileContext(nc) as tc:
        with (
            tc.tile_pool(name="big", bufs=1) as big,
            tc.tile_pool(name="work", bufs=1) as work,
            tc.tile_pool(name="ps", bufs=1, space=PS) as ps,
        ):
            sta_sb = [big.tile([128, w, STA_W], f8, tag=f"sta{p}", name=f"sta_sb{p}")
                      for p, w in enumerate(STA_PIECES)]
            mov_sb = [big.tile([128, MOV_W, HCH], f8, tag=f"mov{h}", name=f"mov_sb{h}")
                      for h in range(NH)]
            nc.sync.dma_start(sta_sb[0][:], sta_d[0][:])
            nc.scalar.dma_start(mov_sb[0][:], mov_d[0][:])
            nc.scalar.dma_start(mov_sb[1][:], mov_d[1][:])
            nc.sync.dma_start(sta_sb[1][:], sta_d[1][:])
            nc.sync.dma_start(sta_sb[2][:], sta_d[2][:])
            nc.sync.dma_start(sta_sb[3][:], sta_d[3][:])
            nc.sync.dma_start(sta_sb[4][:], sta_d[4][:])

            stage = work.tile([128, 1, 1, 512], f32, tag="stage")
            nc.gpsimd.memset(stage[:], 0.0)
            if OUT_MODE == "kv":
                kvidx = work.tile([128, 1], mybir.dt.int32, tag="kvidx")
                nc.gpsimd.memset(kvidx[:], 0)
                kv_sem = nc.alloc_semaphore("kv_dma")
                stage_sem = nc.alloc_semaphore("stage_done")
                nc.gpsimd.kv_writeback(out_d[:],
                                       stage[:].bitcast(mybir.dt.bfloat16),
                                       kvidx[:], prepare_only=True, sem=kv_sem)

            # on-device one-hot expansion: oh[k, ch] = (assign == k)
            oh_sb = [work.tile([128, 10, HCH], f8, tag=f"oh{h}", name=f"oh_sb{h}")
                     for h in range(NH)]
            for h in range(NH):
                for k in range(10):
                    nc.vector.tensor_scalar(
                        oh_sb[h][:, k, :], mov_sb[h][:, 0, :], float(k), None,
                        op0=mybir.AluOpType.is_equal)

            # one PSUM bank per 8-graph quarter so the early drain copies
            # never share a bank with in-flight matmul writes
            psA = [ps.tile([75, 8, 10], f32, tag=f"psA{q}", name=f"psA{q}")
                   for q in range(4)]
            psB = ps.tile([65, 10], f32, tag="psB")

            for c in range(CHUNKS):
                p, cq = piece_of[c]
                h, ch = divmod(c, HCH)
                g = c // 4
                nc.tensor.matmul(psA[g // 8][:, g % 8, :],
                                 sta_sb[p][:, cq, 0:75],
                                 oh_sb[h][:, :, ch],
                                 start=(c % 4 == 0), stop=(c % 4 == 3))
                nc.tensor.matmul(psB[:],
                                 sta_sb[p][:, cq, 75:140],
                                 mov_sb[h][:, 1:11, ch],
                                 start=(c == 0), stop=(c == CHUNKS - 1))
                # drain finished quarters out of PSUM early (ACT engine), so
                # only the last small copy sits on the tail
                if c % 32 == 31:
                    q = c // 32
                    cp = nc.scalar.copy(stage[0:75, 0, 0, q * 80:(q + 1) * 80],
                                        psA[q][:])
                    if q == 3 and OUT_MODE == "kv":
                        last_act_name = cp.ins.name

            # qmax over all chunks: free-axis max-reduce per mov half, then
            # combine the halves straight into the staging tile.
            rmax = [work.tile([128, 10, 1], f32, tag=f"rmax{h}", name=f"rmax{h}")
                    for h in range(NH)]
            for h in range(NH):
                nc.vector.tensor_reduce(rmax[h][:], mov_sb[h][:, 1:11, :],
                                        axis=mybir.AxisListType.X,
                                        op=mybir.AluOpType.max)
            nc.vector.tensor_tensor(stage[:, 0, 0, 320:330], rmax[0][:, :, 0],
                                    rmax[1][:, :, 0], op=mybir.AluOpType.max)
            pbcp = nc.vector.tensor_copy(stage[0:65, 0, 0, 330:340], psB[:])
            last_dve_name = pbcp.ins.name if OUT_MODE == "kv" else None

            if OUT_MODE == "kv":
                trig = nc.gpsimd.trigger_dma(count=None)
            else:
                nc.sync.dma_start(out_d[:], stage[:, 0, 0, :])

    # Tile's epilogue gates on its DMASW-lane semaphore for the kv prep, but
    # the DMA-completion sem baked into the descriptor is the one passed via
    # sem=. Retarget the prep's completion update to the DMASW sem so the
    # increment lands where the epilogue (and data consumers) wait.
    fn = nc.m.functions[0]
    dmasw = None
    kv_inst = None
    trig_inst = None
    if OUT_MODE != "kv":
        nc.compile()
        return nc
    for b in fn.blocks:
        for inst in b.instructions:
            if type(inst).__name__ == "InstKVWritebackAnt":
                kv_inst = inst
            if type(inst).__name__ == "InstTriggerDma":
                trig_inst = inst
            si = inst.sync_info
            if si is not None:
                for w in (si.on_wait or []):
                    if w.ant_name and w.ant_name.startswith("DMASW") \
                            and w.wait_value == 16:
                        dmasw = (w.id, w.ant_name)
    assert kv_inst is not None and dmasw is not None, (kv_inst, dmasw)
    u0 = kv_inst.sync_info.on_update[0]
    assert u0.ant_name == "kv_dma", u0
    u0.id, u0.ant_name = dmasw

    # Drop the WAR waits Tile put on the first ACT/DVE stage writers (they
    # guard against the kv DMA reading stage, but the trigger now orders that
    # read after all writers). Keep the epilogue quiesce in the final block.
    # Gate the trigger on the last stage writer of each engine: wait for the
    # engine-tick semaphore to reach that writer's increment count (counted in
    # engine order, so elided earlier increments cannot desynchronize it).
    assert trig_inst is not None
    new_waits = []
    for want in (last_act_name, last_dve_name):
        count = 0
        sem = None
        done = False
        for b in fn.blocks:
            for inst in b.instructions:
                si = inst.sync_info
                ups = (si.on_update or []) if si else []
                tick = [u for u in ups if u.ant_name and
                        (u.ant_name.startswith("Activation_") or
                         u.ant_name.startswith("DVE_"))]
                if inst.name == want:
                    assert tick, f"writer {want} has no engine tick inc"
                    sem = tick[0]
                    done = True
            if done:
                break
        assert sem is not None
        count = 0
        for b in fn.blocks:
            stop = False
            for inst in b.instructions:
                si = inst.sync_info
                ups = (si.on_update or []) if si else []
                for u in ups:
                    if u.id == sem.id:
                        count += u.update_value if u.update_value else 1
                if inst.name == want:
                    stop = True
                    break
            if stop:
                break
        new_waits.append(mybir.SyncWait(
            sync_type="semaphore", id=sem.id, ant_name=sem.ant_name,
            wait_mode="sem-ge-imm", wait_value=count))
    tsi = trig_inst.sync_info
    tsi.on_wait = list(tsi.on_wait or []) + new_waits

    blocks = list(fn.blocks)
    for b in blocks[:-1]:
        for inst in b.instructions:
            si = inst.sync_info
            if si is None or not si.on_wait:
                continue
            kept = [w for w in si.on_wait
                    if not (w.ant_name and w.ant_name.startswith("DMASW"))]
            if len(kept) != len(si.on_wait):
                si.on_wait = kept

    nc.compile()
    return nc


def _get_program():
    if "nc" not in _CACHE:
        _CACHE["nc"] = _build_program()
    return _CACHE["nc"]


def _host_assign(S):
    """Reproduce jax.random.categorical(key(42), log(S+1e-30)) exactly."""
    import jax
    import jax.numpy as jnp
    cpu = jax.devices("cpu")[0]
    with jax.default_device(cpu):
        a = jax.random.categorical(
            jax.random.key(42), jnp.log(jnp.asarray(S) + 1e-30), axis=-1)
        return np.asarray(a).astype(np.int32)


def _log_softmax(x):
    m = x.max(axis=-1, keepdims=True)
    e = x - m
    return e - np.log(np.exp(e).sum(axis=-1, keepdims=True))


def kernel(Q, E, ind_positive_sample, S, H, L_rows, L_cols, L_vals, batch,
           pred1, pred2, labels):
    import ml_dtypes
    f8 = ml_dtypes.float8_e4m3

    Q = np.asarray(Q, dtype=np.float32)
    E = np.asarray(E, dtype=np.float32)
    S = np.asarray(S, dtype=np.float32)
    H = np.asarray(H, dtype=np.float32)
    L_rows = np.asarray(L_rows)
    L_cols = np.asarray(L_cols)
    L_vals = np.asarray(L_vals, dtype=np.float32)
    pred1 = np.asarray(pred1, dtype=np.float32)
    pred2 = np.asarray(pred2, dtype=np.float32)
    labels = np.asarray(labels).astype(np.int64)

    # host index preprocessing
    assign = _host_assign(S)                        # [N] int32
    Qf = Q.reshape(N, K)
    Ef = E.reshape(N, D)
    rownorm2 = np.einsum('nd,nd->n', H, H)          # [N] f32

    # fp8 staging: sta = [H | S2*64 | rn2/16 | E | 1], mov = [assign | Q*8]
    sta8 = np.empty((N, STA_W), dtype=f8)
    sta8[:, 0:64] = H.astype(f8)
    sta8[:, 64:74] = (S * S * SCALE_S2).astype(f8)
    sta8[:, 74] = (rownorm2 * SCALE_RN).astype(f8)
    sta8[:, 75:139] = Ef.astype(f8)
    sta8[:, 139] = f8(1.0)

    mov8 = np.empty((N, MOV_W), dtype=f8)
    mov8[:, 0] = assign.astype(f8)
    mov8[:, 1:11] = (Qf * SCALE_Q).astype(f8)

    in_maps = []
    for cid in range(NC):
        r0 = cid * N_SH
        m = {}
        c0 = 0
        for p, w in enumerate(STA_PIECES):
            blk = sta8[r0 + c0 * 128: r0 + (c0 + w) * 128]
            m[f"sta{p}_in"] = np.ascontiguousarray(
                blk.reshape(w, 128, STA_W).transpose(1, 0, 2))
            c0 += w
        for h in range(NH):
            blk = mov8[r0 + h * HCH * 128: r0 + (h + 1) * HCH * 128]
            m[f"mov{h}_in"] = np.ascontiguousarray(
                blk.reshape(HCH, 128, MOV_W).transpose(1, 2, 0))
        in_maps.append(m)

    nc = _get_program()
    from concourse.bass_utils import run_bass_kernel_spmd
    res = run_bass_kernel_spmd(nc, in_maps, core_ids=list(range(NC)))
    outs = res.results
    _CACHE["last_exec_time_ns"] = res.exec_time_ns

    # ---- reassemble device outputs ----
    bvec = np.asarray(batch).astype(np.int64)
    counts = np.bincount(bvec * K + assign, minlength=B * K).reshape(B, K)
    counts = counts.astype(np.float32)
    sums = np.zeros((B, K, D), dtype=np.float32)
    colnorm2 = np.zeros((B, K), dtype=np.float32)
    rn2s = np.zeros((B, K), dtype=np.float32)
    proto_sum = np.zeros((K, D), dtype=np.float32)
    q_count = np.zeros((K,), dtype=np.float32)
    qmax = np.full((K,), -np.inf, dtype=np.float32)
    for cid in range(NC):
        raw = np.asarray(outs[cid]["out"])
        if raw.dtype != np.float32:      # kv path returns a bf16-viewed buffer
            raw = np.ascontiguousarray(raw).view(np.uint16).view(np.float32)
        out = raw.reshape(128, 512).astype(np.float32)
        ft = out[0:75, 0:320].reshape(75, G_SH, 10)
        g0 = cid * G_SH
        sums[g0:g0 + G_SH] = ft[0:64].transpose(1, 2, 0)
        colnorm2[g0:g0 + G_SH] = ft[64:74].sum(axis=2).T / SCALE_S2
        rn2s[g0:g0 + G_SH] = ft[74] / SCALE_RN
        proto_sum += out[0:64, 330:340].T / SCALE_Q
        q_count += out[64, 330:340] / SCALE_Q
        qmax = np.maximum(qmax, out[:, 320:330].max(axis=0) / SCALE_Q)

    # ---- loss_1 / loss_2 ----
    ls1 = _log_softmax(pred1)
    loss_1 = -np.mean(ls1[np.arange(B), labels])
    ls2 = _log_softmax(pred2)
    ce2 = -ls2[np.arange(B), labels]
    mask = np.asarray(ind_positive_sample).astype(np.float32)
    npos = mask.sum()
    loss_2 = LAMBDA_2 * (float((mask * ce2).sum()) / max(npos, 1.0) if npos > 0 else 0.0)

    # ---- connectivity ----
    colnorm = np.sqrt(colnorm2)
    S_n = S / (colnorm[bvec] + 1e-5)
    loss_sp = 0.0
    CH = 1 << 19
    for i in range(0, NNZ, CH):
        r = L_rows[i:i + CH].astype(np.int64)
        c = L_cols[i:i + CH].astype(np.int64)
        v = L_vals[i:i + CH]
        loss_sp += float((v * np.einsum('ek,ek->e', S_n[r], S_n[c])).sum())
    ss = S_n.T @ S_n
    i_s = np.eye(K, dtype=np.float32) * B
    loss_ortho = float(np.sqrt(((ss - i_s) ** 2).sum()))
    con = LAMBDA_CON * (loss_sp + loss_ortho) / B

    # ---- feature loss ----
    cmax = np.maximum(counts, 1.0)
    means = sums / cmax[..., None]
    sq_tot = rn2s - (2.0 * means * sums).sum(-1) + counts * (means * means).sum(-1)
    fd = sq_tot / D
    feature_loss = float(np.where(counts > 0, fd / cmax, 0.0).sum())
    pd = ((means[:, :, None, :] - means[:, None, :, :]) ** 2).mean(axis=-1)
    c_g = 0.5 * pd.sum(axis=(1, 2))
    center = 0.0
    for i in range(B):
        center = (center - float(c_g[i])) / (K - 1)
    fea = LAMBDA_FEA * (feature_loss + center) / B

    # ---- prototype loss ----
    loss1 = float(np.mean(1.0 - qmax))
    proto = proto_sum / (q_count + 0.1)[:, None]
    proto = proto / (np.linalg.norm(proto, axis=1) + 1e-15)[:, None]
    pdist = ((proto[:, None, :] - proto[None, :, :]) ** 2).mean(axis=-1)
    center_loss = -0.5 * float(pdist.sum()) / (K * (K - 1) / 2)
    proto_l = LAMBDA_PROTO * (loss1 + center_loss)

    total = loss_1 + loss_2 + con + fea + proto_l
    return np.float32(total)
